# revision 22
# baseline (speedup 1.0000x reference)
"""YOLO-style loss kernel for Trainium2, SPMD over 8 NeuronCores (v2).

Inputs (full): pred_tensor [32768,7,7,30] f32, target_tensor [32768,7,7,30] f32.
Output: np.ndarray shape (5,) f32 = (loss_xy, loss_wh, loss_obj, loss_noobj, loss_class).

v2 strategy (calibrated against real HW instruction costs):
- Host converts to bf16 (halves HBM traffic; rel err ~1e-4, gate is 2e-2) and
  permutes channels per box-coordinate type so every device op is a
  contiguous <=3D access pattern:
    pred  30 cols: [x0,y0,x1,y1 | w0,h0,w1,h1 | c0,c1 | 20 class]
    tgt   38 cols: [same 30 | tx0,ty0,tx0,ty0 | tw0,th0,tw0,th0]  (box0 dup)
- Host pre-scales xy by 1/7 and wh by 1/2 (constant linear transform) so the
  ltrb corners are plain adds/subs (packed-bf16 TensorTensor at 2x rate
  instead of ScalarTensorTensor at 1x). Final scalars fixed up on host
  (loss_xy *= 49, loss_wh *= 2); IoU is scale-consistent (areas via *4).
- Activation engine does sqrt/relu/reciprocal (raw InstActivation Reciprocal,
  ~1e-5 rel err) and one big Square over the packed 32-col diff tile.
- loss_noobj uses noobj*(t_c - p_c)^2 == noobj*p_c^2 exactly (t_c == 0 under
  noobj), so the raw conf diffs double as the noobj-loss diffs.
- Seven masked accum reductions (xy_b0, xy_b1, wh_b0, wh_b1, obj, noobj,
  class) as DVE scalar_tensor_tensor with accum_out, masks broadcast via
  step-0 access patterns. Host sums partitions/chunks and rescales.
"""

import os
import sys

sys.path.insert(0, "/opt/trn_rl_repo")

import numpy as np

import concourse.bass as bass
import concourse.bacc as bacc
import concourse.tile as tile
from concourse import mybir
from concourse import bass_utils

F32 = mybir.dt.float32
BF16 = mybir.dt.bfloat16
ALU = mybir.AluOpType
ACT = mybir.ActivationFunctionType

S = 7
B = 2
C = 20
D = 30
DT = 38                                # permuted target cols (incl box0 l/r)
DP = 30                                # permuted pred cols
N_FULL = 32768
N_CORES = 8
N_SHARD = N_FULL // N_CORES            # 4096 samples per core
R = N_SHARD * S * S                    # 200704 cells per core
P = 128                                # partitions
RP = R // P                            # 1568 cells per partition
N_CHUNK = 392                          # cells per partition per chunk
N_CH = RP // N_CHUNK                   # 4 chunks
NLOSS = 7                              # accum columns per chunk


def _expand(ap, pos, count):
    """Insert a broadcast (step 0) dim at position `pos` of the ap list."""
    new = [list(x) for x in ap.ap]
    new.insert(pos, [0, count])
    return bass.AP(tensor=ap.tensor, offset=ap.offset, ap=new)


def _act_recip(nc, out, in_):
    """Raw Activation Reciprocal (bass guards it off; ~1e-5 rel err, fine here)."""
    nc.scalar.add_instruction(
        mybir.InstActivation(
            name=nc.get_next_instruction_name(),
            func=ACT.Reciprocal,
            ins=[
                nc.scalar.lower_ap(in_),
                mybir.ImmediateValue(dtype=mybir.dt.float32, value=0.0),
                mybir.ImmediateValue(dtype=mybir.dt.float32, value=1.0),
                mybir.ImmediateValue(dtype=mybir.dt.float32, value=0.0),
            ],
            outs=[nc.scalar.lower_ap(out)],
        )
    )


def build_program(rep=1):
    nc = bacc.Bacc("TRN2", target_bir_lowering=False, debug=False)

    pred = nc.dram_tensor("pred", [R, DP], BF16, kind="ExternalInput")
    tgt = nc.dram_tensor("tgt", [R, DT], BF16, kind="ExternalInput")
    out = nc.dram_tensor("out", [P, NLOSS * N_CH], F32, kind="ExternalOutput")

    pred_v = pred.ap().rearrange("(p r) c -> p r c", p=P)
    tgt_v = tgt.ap().rearrange("(p r) c -> p r c", p=P)

    n = N_CHUNK
    with tile.TileContext(nc) as tc:
        with (
            tc.tile_pool(name="raw", bufs=2) as raw,
            tc.tile_pool(name="tmp", bufs=1) as tmp,
            tc.tile_pool(name="persist", bufs=1) as persist,
        ):
            acc = persist.tile([P, NLOSS * N_CH], F32)
            nc.vector.memset(acc, 0.0)   # cols 1,3 unused by merged reductions

            for k in [k for _ in range(rep) for k in range(N_CH)]:
                Pt = raw.tile([P, n, DP], BF16, tag="P")
                Tt = raw.tile([P, n, DT], BF16, tag="T")
                nc.sync.dma_start(out=Pt, in_=pred_v[:, k * n:(k + 1) * n, :])
                nc.sync.dma_start(out=Tt, in_=tgt_v[:, k * n:(k + 1) * n, :])

                obj = Tt[:, :, 8]                    # {0,1} exact

                # --- pred-only ops first: they unblock as soon as the Pt DMA
                # lands, hiding the (later-queued) Tt DMA latency ---
                lp4 = tmp.tile([P, n, 4], BF16, tag="lp4")
                rp4 = tmp.tile([P, n, 4], BF16, tag="rp4")
                ap2 = tmp.tile([P, n, 2], BF16, tag="ap2")
                sqp4 = tmp.tile([P, n, 4], BF16, tag="sqp4")
                nc.vector.tensor_tensor(lp4, Pt[:, :, 0:4], Pt[:, :, 4:8], op=ALU.subtract)
                nc.vector.tensor_tensor(rp4, Pt[:, :, 0:4], Pt[:, :, 4:8], op=ALU.add)
                nc.vector.scalar_tensor_tensor(ap2, Pt[:, :, 4:8:2], 4.0, Pt[:, :, 5:8:2], op0=ALU.mult, op1=ALU.mult)
                nc.scalar.activation(sqp4, Pt[:, :, 4:8], ACT.Sqrt)

                # d32 cols: 0:4 xy, 4:8 sqrt-wh, 8:10 raw conf (noobj),
                # 10:30 class, 30:32 riou-conf
                d32 = tmp.tile([P, n, 32], BF16, tag="d32")
                nc.vector.tensor_tensor(d32[:, :, 0:4], Tt[:, :, 0:4], Pt[:, :, 0:4], op=ALU.subtract)
                nc.vector.tensor_tensor(d32[:, :, 8:30], Tt[:, :, 8:30], Pt[:, :, 8:30], op=ALU.subtract)

                # sqrt(wh/2) diffs -> d32[4:8]
                sqt4 = tmp.tile([P, n, 4], BF16, tag="sqt4")
                nc.scalar.activation(sqt4, Tt[:, :, 4:8], ACT.Sqrt)
                nc.vector.tensor_tensor(d32[:, :, 4:8], sqt4, sqp4, op=ALU.subtract)

                # IoU (pred boxes b0,b1 vs tgt box0; [n,4] packed bf16)
                lt4 = tmp.tile([P, n, 4], BF16, tag="lt4")
                rt4 = tmp.tile([P, n, 4], BF16, tag="rt4")
                nc.vector.tensor_tensor(lt4, Tt[:, :, 30:34], Tt[:, :, 34:38], op=ALU.subtract)
                nc.vector.tensor_tensor(rt4, Tt[:, :, 30:34], Tt[:, :, 34:38], op=ALU.add)

                nc.vector.tensor_tensor(lp4, lp4, lt4, op=ALU.max)       # ml4 in place
                nc.vector.tensor_tensor(rp4, rp4, rt4, op=ALU.min)       # mr4
                nc.vector.tensor_tensor(lt4, rp4, lp4, op=ALU.subtract)  # dw4
                cw4 = rt4
                nc.scalar.activation(cw4, lt4, ACT.Relu)

                inter2 = tmp.tile([P, n, 2], BF16, tag="inter2")
                nc.vector.tensor_tensor(inter2, cw4[:, :, 0:4:2], cw4[:, :, 1:4:2], op=ALU.mult)

                at1 = tmp.tile([P, n], BF16, tag="at1")
                nc.vector.scalar_tensor_tensor(at1, Tt[:, :, 34], 4.0, Tt[:, :, 35], op0=ALU.mult, op1=ALU.mult)

                su2 = tmp.tile([P, n, 2], BF16, tag="su2")
                un2 = tmp.tile([P, n, 2], BF16, tag="un2")
                nc.vector.tensor_tensor(su2, ap2, _expand(at1[:, :], 2, 2), op=ALU.add)
                nc.vector.tensor_tensor(un2, su2, inter2, op=ALU.subtract)

                rcp2 = tmp.tile([P, n, 2], F32, tag="rcp2")
                _act_recip(nc, rcp2, un2)
                iou2 = su2  # dead, reuse
                nc.vector.tensor_tensor(iou2, inter2, rcp2, op=ALU.mult)

                is1 = tmp.tile([P, n], BF16, tag="is1")
                riou = tmp.tile([P, n], BF16, tag="riou")
                resp = tmp.tile([P, n, 2], BF16, tag="resp")
                nc.vector.tensor_tensor(is1, iou2[:, :, 1], iou2[:, :, 0], op=ALU.is_gt)
                nc.vector.tensor_tensor(riou, iou2[:, :, 1], iou2[:, :, 0], op=ALU.max)
                nc.vector.tensor_tensor(resp[:, :, 1], obj, is1, op=ALU.mult)
                nc.vector.tensor_tensor(resp[:, :, 0], obj, resp[:, :, 1], op=ALU.subtract)

                # dcf2 = riou - p_conf -> d32[30:32]
                nc.vector.scalar_tensor_tensor(
                    d32[:, :, 30:32], Pt[:, :, 8:10], -1.0, _expand(riou[:, :], 2, 2),
                    op0=ALU.mult, op1=ALU.add,
                )

                # noobj mask (Act engine has slack; Copy is in every table set)
                nobj = tmp.tile([P, n], BF16, tag="nobj")
                nc.scalar.activation(nobj, obj, ACT.Copy, bias=1.0, scale=-1.0)

                # squares split in two halves so the first reductions overlap
                # only the second half of the Act work
                sq32 = tmp.tile([P, n, 32], BF16, tag="sq32")
                nc.scalar.activation(sq32[:, :, 0:8], d32[:, :, 0:8], ACT.Square)

                a0 = NLOSS * k

                def red(in0, in1, col, junk):
                    nc.vector.scalar_tensor_tensor(
                        junk, in0, 1.0, in1, op0=ALU.mult, op1=ALU.mult,
                        accum_out=acc[:, a0 + col:a0 + col + 1],
                    )

                # resp4 = (r0,r0,r1,r1) via two tiny Act copies -> xy and wh
                # reductions collapse to one STT each (saves 2 DVE ops/chunk)
                resp4 = tmp.tile([P, n, 4], BF16, tag="resp4")
                nc.scalar.activation(resp4[:, :, 0:2], _expand(resp[:, :, 0], 2, 2), ACT.Copy)
                nc.scalar.activation(resp4[:, :, 2:4], _expand(resp[:, :, 1], 2, 2), ACT.Copy)
                red(sq32[:, :, 0:4], resp4, 0, d32[:, :, 0:4])                           # xy b0+b1
                red(sq32[:, :, 4:8], resp4, 2, d32[:, :, 4:8])                           # wh b0+b1

                nc.scalar.activation(sq32[:, :, 8:32], d32[:, :, 8:32], ACT.Square)

                red(sq32[:, :, 30:32], resp, 4, d32[:, :, 30:32])                        # obj
                red(sq32[:, :, 8:10], _expand(nobj[:, :], 2, 2), 5, d32[:, :, 8:10])     # noobj
                # class: per-cell rowsum (TensorReduce, bf16) then tiny masked accum;
                # a direct [n,20] STT runs at 1 elem/cycle with no bf16 speedup.
                csum = tmp.tile([P, n], BF16, tag="csum")
                with nc.allow_low_precision("class rowsum; cross-cell accum stays f32"):
                    nc.vector.tensor_reduce(csum, sq32[:, :, 10:30], axis=mybir.AxisListType.X, op=ALU.add)
                red(csum, obj, 6, d32[:, :, 10])                                         # class

            nc.sync.dma_start(out=out.ap(), in_=acc)

    nc.compile()
    return nc


_nc_cache = None
LAST_EXEC_NS = None
LAST_RESULT = None


def _get_nc():
    global _nc_cache
    if _nc_cache is None:
        _nc_cache = build_program(rep=int(os.environ.get("KERNEL_REP", "1")))
    return _nc_cache


# permutation for pred cols: [x0,y0,x1,y1, w0,h0,w1,h1, c0,c1, class...]
_PERM = [0, 1, 5, 6, 2, 3, 7, 8, 4, 9] + list(range(10, 30))
_XY_COLS = [0, 1, 2, 3]
_WH_COLS = [4, 5, 6, 7]


def make_in_maps(pred_tensor, target_tensor):
    import ml_dtypes

    bf16 = ml_dtypes.bfloat16
    pred = np.asarray(pred_tensor, dtype=np.float32).reshape(N_FULL * S * S, D)
    tgt = np.asarray(target_tensor, dtype=np.float32).reshape(N_FULL * S * S, D)

    pp = np.empty((N_FULL * S * S, DP), np.float32)
    pp[:, :] = pred[:, _PERM]
    tp = np.empty((N_FULL * S * S, DT), np.float32)
    tp[:, :30] = tgt[:, _PERM]
    # scale xy by 1/7, wh by 1/2 (losses rescaled on host afterwards)
    for a in (pp, tp):
        a[:, _XY_COLS] *= 1.0 / 7.0
        a[:, _WH_COLS] *= 0.5
    # tgt box0 duplicated (already scaled)
    tp[:, 30] = tp[:, 0]
    tp[:, 31] = tp[:, 1]
    tp[:, 32] = tp[:, 0]
    tp[:, 33] = tp[:, 1]
    tp[:, 34] = tp[:, 4]
    tp[:, 35] = tp[:, 5]
    tp[:, 36] = tp[:, 4]
    tp[:, 37] = tp[:, 5]

    pp16 = pp.astype(bf16)
    tp16 = tp.astype(bf16)

    in_maps = []
    for i in range(N_CORES):
        lo, hi = i * R, (i + 1) * R
        in_maps.append({"pred": pp16[lo:hi], "tgt": tp16[lo:hi]})
    return in_maps


def reduce_out_maps(out_maps):
    total = np.zeros(NLOSS, dtype=np.float64)
    for m in out_maps:
        total += m["out"].astype(np.float64).sum(axis=0).reshape(N_CH, NLOSS).sum(axis=0)
    n = float(N_FULL)
    loss_xy = (total[0] + total[1]) * 49.0 / n
    loss_wh = (total[2] + total[3]) * 2.0 / n
    loss_obj = total[4] / n
    loss_noobj = total[5] / n
    loss_class = total[6] / n
    return np.asarray([loss_xy, loss_wh, loss_obj, loss_noobj, loss_class], dtype=np.float32)


def kernel(pred_tensor, target_tensor):
    global LAST_EXEC_NS, LAST_RESULT
    in_maps = make_in_maps(pred_tensor, target_tensor)

    nc = _get_nc()
    trace = os.environ.get("KERNEL_TRACE", "") not in ("", "0")
    res = bass_utils.run_bass_kernel_spmd(
        nc, in_maps, core_ids=list(range(N_CORES)), trace=trace
    )
    LAST_RESULT = res
    if res.exec_time_ns is not None:
        LAST_EXEC_NS = res.exec_time_ns
    return reduce_out_maps(res.results)


# revision 23
# speedup vs baseline: 1.1906x; 1.1906x over previous
"""YOLO-style loss kernel for Trainium2, SPMD over 8 NeuronCores (v2).

Inputs (full): pred_tensor [32768,7,7,30] f32, target_tensor [32768,7,7,30] f32.
Output: np.ndarray shape (5,) f32 = (loss_xy, loss_wh, loss_obj, loss_noobj, loss_class).

v2 strategy (calibrated against real HW instruction costs):
- Host converts to bf16 (halves HBM traffic; rel err ~1e-4, gate is 2e-2) and
  permutes channels per box-coordinate type so every device op is a
  contiguous <=3D access pattern:
    pred  30 cols: [x0,y0,x1,y1 | w0,h0,w1,h1 | c0,c1 | 20 class]
    tgt   38 cols: [same 30 | tx0,ty0,tx0,ty0 | tw0,th0,tw0,th0]  (box0 dup)
- Host pre-scales xy by 1/7 and wh by 1/2 (constant linear transform) so the
  ltrb corners are plain adds/subs (packed-bf16 TensorTensor at 2x rate
  instead of ScalarTensorTensor at 1x). Final scalars fixed up on host
  (loss_xy *= 49, loss_wh *= 2); IoU is scale-consistent (areas via *4).
- Activation engine does sqrt/relu/reciprocal (raw InstActivation Reciprocal,
  ~1e-5 rel err) and one big Square over the packed 32-col diff tile.
- loss_noobj uses noobj*(t_c - p_c)^2 == noobj*p_c^2 exactly (t_c == 0 under
  noobj), so the raw conf diffs double as the noobj-loss diffs.
- Seven masked accum reductions (xy_b0, xy_b1, wh_b0, wh_b1, obj, noobj,
  class) as DVE scalar_tensor_tensor with accum_out, masks broadcast via
  step-0 access patterns. Host sums partitions/chunks and rescales.
"""

import os
import sys

sys.path.insert(0, "/opt/trn_rl_repo")

import numpy as np

import concourse.bass as bass
import concourse.bacc as bacc
import concourse.tile as tile
from concourse import mybir
from concourse import bass_utils

F32 = mybir.dt.float32
BF16 = mybir.dt.bfloat16
ALU = mybir.AluOpType
ACT = mybir.ActivationFunctionType

S = 7
B = 2
C = 20
D = 30
DT = 38                                # permuted target cols (incl box0 l/r)
DP = 30                                # permuted pred cols
N_FULL = 32768
N_CORES = 8
N_SHARD = N_FULL // N_CORES            # 4096 samples per core
R = N_SHARD * S * S                    # 200704 cells per core
P = 128                                # partitions
RP = R // P                            # 1568 cells per partition
N_CHUNK = 392                          # cells per partition per chunk
N_CH = RP // N_CHUNK                   # 4 chunks
NLOSS = 7                              # accum columns per chunk


def _expand(ap, pos, count):
    """Insert a broadcast (step 0) dim at position `pos` of the ap list."""
    new = [list(x) for x in ap.ap]
    new.insert(pos, [0, count])
    return bass.AP(tensor=ap.tensor, offset=ap.offset, ap=new)


def _act_recip(nc, out, in_):
    """Raw Activation Reciprocal (bass guards it off; ~1e-5 rel err, fine here)."""
    nc.scalar.add_instruction(
        mybir.InstActivation(
            name=nc.get_next_instruction_name(),
            func=ACT.Reciprocal,
            ins=[
                nc.scalar.lower_ap(in_),
                mybir.ImmediateValue(dtype=mybir.dt.float32, value=0.0),
                mybir.ImmediateValue(dtype=mybir.dt.float32, value=1.0),
                mybir.ImmediateValue(dtype=mybir.dt.float32, value=0.0),
            ],
            outs=[nc.scalar.lower_ap(out)],
        )
    )


def build_program(rep=1):
    nc = bacc.Bacc("TRN2", target_bir_lowering=False, debug=False)

    pred = nc.dram_tensor("pred", [R, DP], BF16, kind="ExternalInput")
    tgt = nc.dram_tensor("tgt", [R, DT], BF16, kind="ExternalInput")
    out = nc.dram_tensor("out", [P, NLOSS * N_CH], F32, kind="ExternalOutput")

    pred_v = pred.ap().rearrange("(p r) c -> p r c", p=P)
    tgt_v = tgt.ap().rearrange("(p r) c -> p r c", p=P)

    n = N_CHUNK
    with tile.TileContext(nc) as tc:
        with (
            tc.tile_pool(name="raw", bufs=2) as raw,
            tc.tile_pool(name="tmp", bufs=1) as tmp,
            tc.tile_pool(name="persist", bufs=1) as persist,
        ):
            acc = persist.tile([P, NLOSS * N_CH], F32)

            for k in [k for _ in range(rep) for k in range(N_CH)]:
                Pt = raw.tile([P, n, DP], BF16, tag="P")
                Tt = raw.tile([P, n, DT], BF16, tag="T")
                nc.sync.dma_start(out=Pt, in_=pred_v[:, k * n:(k + 1) * n, :])
                nc.sync.dma_start(out=Tt, in_=tgt_v[:, k * n:(k + 1) * n, :])

                obj = Tt[:, :, 8]                    # {0,1} exact

                # --- pred-only ops first: they unblock as soon as the Pt DMA
                # lands, hiding the (later-queued) Tt DMA latency ---
                lp4 = tmp.tile([P, n, 4], BF16, tag="lp4")
                rp4 = tmp.tile([P, n, 4], BF16, tag="rp4")
                ap2 = tmp.tile([P, n, 2], BF16, tag="ap2")
                sqp4 = tmp.tile([P, n, 4], BF16, tag="sqp4")
                nc.vector.tensor_tensor(lp4, Pt[:, :, 0:4], Pt[:, :, 4:8], op=ALU.subtract)
                nc.vector.tensor_tensor(rp4, Pt[:, :, 0:4], Pt[:, :, 4:8], op=ALU.add)
                nc.vector.scalar_tensor_tensor(ap2, Pt[:, :, 4:8:2], 4.0, Pt[:, :, 5:8:2], op0=ALU.mult, op1=ALU.mult)
                nc.scalar.activation(sqp4, Pt[:, :, 4:8], ACT.Sqrt)

                # d32 cols: 0:4 xy, 4:8 sqrt-wh, 8:10 raw conf (noobj),
                # 10:30 class, 30:32 riou-conf
                d32 = tmp.tile([P, n, 32], BF16, tag="d32")
                nc.vector.tensor_tensor(d32[:, :, 0:4], Tt[:, :, 0:4], Pt[:, :, 0:4], op=ALU.subtract)
                nc.vector.tensor_tensor(d32[:, :, 8:30], Tt[:, :, 8:30], Pt[:, :, 8:30], op=ALU.subtract)

                # sqrt(wh/2) diffs -> d32[4:8]
                sqt4 = tmp.tile([P, n, 4], BF16, tag="sqt4")
                nc.scalar.activation(sqt4, Tt[:, :, 4:8], ACT.Sqrt)
                nc.vector.tensor_tensor(d32[:, :, 4:8], sqt4, sqp4, op=ALU.subtract)

                # IoU (pred boxes b0,b1 vs tgt box0; [n,4] packed bf16)
                lt4 = tmp.tile([P, n, 4], BF16, tag="lt4")
                rt4 = tmp.tile([P, n, 4], BF16, tag="rt4")
                nc.vector.tensor_tensor(lt4, Tt[:, :, 30:34], Tt[:, :, 34:38], op=ALU.subtract)
                nc.vector.tensor_tensor(rt4, Tt[:, :, 30:34], Tt[:, :, 34:38], op=ALU.add)

                nc.vector.tensor_tensor(lp4, lp4, lt4, op=ALU.max)       # ml4 in place
                nc.vector.tensor_tensor(rp4, rp4, rt4, op=ALU.min)       # mr4
                nc.vector.tensor_tensor(lt4, rp4, lp4, op=ALU.subtract)  # dw4
                cw4 = rt4
                nc.scalar.activation(cw4, lt4, ACT.Relu)

                inter2 = tmp.tile([P, n, 2], BF16, tag="inter2")
                nc.vector.tensor_tensor(inter2, cw4[:, :, 0:4:2], cw4[:, :, 1:4:2], op=ALU.mult)

                at1 = tmp.tile([P, n], BF16, tag="at1")
                nc.vector.scalar_tensor_tensor(at1, Tt[:, :, 34], 4.0, Tt[:, :, 35], op0=ALU.mult, op1=ALU.mult)

                su2 = tmp.tile([P, n, 2], BF16, tag="su2")
                un2 = tmp.tile([P, n, 2], BF16, tag="un2")
                nc.vector.tensor_tensor(su2, ap2, _expand(at1[:, :], 2, 2), op=ALU.add)
                nc.vector.tensor_tensor(un2, su2, inter2, op=ALU.subtract)

                rcp2 = tmp.tile([P, n, 2], F32, tag="rcp2")
                _act_recip(nc, rcp2, un2)
                iou2 = su2  # dead, reuse
                nc.vector.tensor_tensor(iou2, inter2, rcp2, op=ALU.mult)

                is1 = tmp.tile([P, n], BF16, tag="is1")
                riou = tmp.tile([P, n], BF16, tag="riou")
                resp = tmp.tile([P, n, 2], BF16, tag="resp")
                nc.vector.tensor_tensor(is1, iou2[:, :, 1], iou2[:, :, 0], op=ALU.is_gt)
                nc.vector.tensor_tensor(riou, iou2[:, :, 1], iou2[:, :, 0], op=ALU.max)
                nc.vector.tensor_tensor(resp[:, :, 1], obj, is1, op=ALU.mult)
                nc.vector.tensor_tensor(resp[:, :, 0], obj, resp[:, :, 1], op=ALU.subtract)

                # dcf2 = riou - p_conf -> d32[30:32]
                nc.vector.scalar_tensor_tensor(
                    d32[:, :, 30:32], Pt[:, :, 8:10], -1.0, _expand(riou[:, :], 2, 2),
                    op0=ALU.mult, op1=ALU.add,
                )

                # noobj mask (Act engine has slack; Copy is in every table set)
                nobj = tmp.tile([P, n], BF16, tag="nobj")
                nc.scalar.activation(nobj, obj, ACT.Copy, bias=1.0, scale=-1.0)

                # squares split in two halves so the first reductions overlap
                # only the second half of the Act work
                sq32 = tmp.tile([P, n, 32], BF16, tag="sq32")
                nc.scalar.activation(sq32[:, :, 0:8], d32[:, :, 0:8], ACT.Square)

                a0 = NLOSS * k

                def red(in0, in1, col, junk):
                    nc.vector.scalar_tensor_tensor(
                        junk, in0, 1.0, in1, op0=ALU.mult, op1=ALU.mult,
                        accum_out=acc[:, a0 + col:a0 + col + 1],
                    )

                red(sq32[:, :, 0:2], _expand(resp[:, :, 0], 2, 2), 0, d32[:, :, 0:2])    # xy b0
                red(sq32[:, :, 2:4], _expand(resp[:, :, 1], 2, 2), 1, d32[:, :, 2:4])    # xy b1
                red(sq32[:, :, 4:6], _expand(resp[:, :, 0], 2, 2), 2, d32[:, :, 4:6])    # wh b0
                red(sq32[:, :, 6:8], _expand(resp[:, :, 1], 2, 2), 3, d32[:, :, 6:8])    # wh b1

                nc.scalar.activation(sq32[:, :, 8:32], d32[:, :, 8:32], ACT.Square)

                red(sq32[:, :, 30:32], resp, 4, d32[:, :, 30:32])                        # obj
                red(sq32[:, :, 8:10], _expand(nobj[:, :], 2, 2), 5, d32[:, :, 8:10])     # noobj
                # class: per-cell rowsum (TensorReduce, bf16) then tiny masked accum;
                # a direct [n,20] STT runs at 1 elem/cycle with no bf16 speedup.
                csum = tmp.tile([P, n], BF16, tag="csum")
                with nc.allow_low_precision("class rowsum; cross-cell accum stays f32"):
                    nc.vector.tensor_reduce(csum, sq32[:, :, 10:30], axis=mybir.AxisListType.X, op=ALU.add)
                red(csum, obj, 6, d32[:, :, 10])                                         # class

            nc.sync.dma_start(out=out.ap(), in_=acc)

    nc.compile()
    return nc


_nc_cache = None
LAST_EXEC_NS = None
LAST_RESULT = None


def _get_nc():
    global _nc_cache
    if _nc_cache is None:
        _nc_cache = build_program(rep=int(os.environ.get("KERNEL_REP", "1")))
    return _nc_cache


# permutation for pred cols: [x0,y0,x1,y1, w0,h0,w1,h1, c0,c1, class...]
_PERM = [0, 1, 5, 6, 2, 3, 7, 8, 4, 9] + list(range(10, 30))
_XY_COLS = [0, 1, 2, 3]
_WH_COLS = [4, 5, 6, 7]


def make_in_maps(pred_tensor, target_tensor):
    import ml_dtypes

    bf16 = ml_dtypes.bfloat16
    pred = np.asarray(pred_tensor, dtype=np.float32).reshape(N_FULL * S * S, D)
    tgt = np.asarray(target_tensor, dtype=np.float32).reshape(N_FULL * S * S, D)

    pp = np.empty((N_FULL * S * S, DP), np.float32)
    pp[:, :] = pred[:, _PERM]
    tp = np.empty((N_FULL * S * S, DT), np.float32)
    tp[:, :30] = tgt[:, _PERM]
    # scale xy by 1/7, wh by 1/2 (losses rescaled on host afterwards)
    for a in (pp, tp):
        a[:, _XY_COLS] *= 1.0 / 7.0
        a[:, _WH_COLS] *= 0.5
    # tgt box0 duplicated (already scaled)
    tp[:, 30] = tp[:, 0]
    tp[:, 31] = tp[:, 1]
    tp[:, 32] = tp[:, 0]
    tp[:, 33] = tp[:, 1]
    tp[:, 34] = tp[:, 4]
    tp[:, 35] = tp[:, 5]
    tp[:, 36] = tp[:, 4]
    tp[:, 37] = tp[:, 5]

    pp16 = pp.astype(bf16)
    tp16 = tp.astype(bf16)

    in_maps = []
    for i in range(N_CORES):
        lo, hi = i * R, (i + 1) * R
        in_maps.append({"pred": pp16[lo:hi], "tgt": tp16[lo:hi]})
    return in_maps


def reduce_out_maps(out_maps):
    total = np.zeros(NLOSS, dtype=np.float64)
    for m in out_maps:
        total += m["out"].astype(np.float64).sum(axis=0).reshape(N_CH, NLOSS).sum(axis=0)
    n = float(N_FULL)
    loss_xy = (total[0] + total[1]) * 49.0 / n
    loss_wh = (total[2] + total[3]) * 2.0 / n
    loss_obj = total[4] / n
    loss_noobj = total[5] / n
    loss_class = total[6] / n
    return np.asarray([loss_xy, loss_wh, loss_obj, loss_noobj, loss_class], dtype=np.float32)


def kernel(pred_tensor, target_tensor):
    global LAST_EXEC_NS, LAST_RESULT
    in_maps = make_in_maps(pred_tensor, target_tensor)

    nc = _get_nc()
    trace = os.environ.get("KERNEL_TRACE", "") not in ("", "0")
    res = bass_utils.run_bass_kernel_spmd(
        nc, in_maps, core_ids=list(range(N_CORES)), trace=trace
    )
    LAST_RESULT = res
    if res.exec_time_ns is not None:
        LAST_EXEC_NS = res.exec_time_ns
    return reduce_out_maps(res.results)
